# revision 11
# baseline (speedup 1.0000x reference)
"""Trainium2 Bass kernel for nn_GatingNetwork (moe_routing).

Computes: h = x@W1 + b1; LayerNorm(h)*ln_w + ln_b; GELU(exact); logits = g@W2 + b2;
top-2 sparse softmax -> weights [B, E]; Switch load-balance loss (scalar).

Sharding: data-parallel over tokens across 8 NeuronCores (8192 tokens/core).
x is transposed on the host so the contraction dim (D) lands on SBUF partitions.
"""

import sys
from contextlib import ExitStack

import numpy as np

sys.path.insert(0, "/opt/trn_rl_repo")

import concourse.bacc as bacc
import concourse.tile as tile
from concourse import mybir
from concourse.bass_utils import run_bass_kernel_spmd

B, D, H, E = 65536, 1024, 512, 8
N_CORES = 8
BT = B // N_CORES          # tokens per core = 8192
TILE = 128                 # tokens per mm1 tile
GRP = 2                    # tiles per group (rstd batch + mm2 free dim = 256)
NT = BT // TILE            # 64 tiles
NG = NT // GRP             # 32 groups
KD = D // 128              # 8 contraction chunks
KH = H // 128              # 4 contraction chunks for mm2
LN_EPS = 1e-5
LB_WEIGHT = 0.01

F32 = mybir.dt.float32
F32R = mybir.dt.float32r
I32 = mybir.dt.int32
AF = mybir.ActivationFunctionType
OP = mybir.AluOpType
AXX = mybir.AxisListType.X

_cache = {}


def _build(flags):
    b1nz, lnwnz, lnbnz = flags
    nc = bacc.Bacc("TRN2", target_bir_lowering=False, debug=False,
                   num_devices=N_CORES)

    xt = nc.declare_dram_parameter("xt", [D, BT], F32R, isOutput=False)
    w1d = nc.declare_dram_parameter("w1", [D, H], F32R, isOutput=False)
    w2d = nc.declare_dram_parameter("w2", [H, E], F32R, isOutput=False)
    b2d = nc.declare_dram_parameter("b2", [E, 1], F32, isOutput=False)
    if b1nz:
        b1d = nc.declare_dram_parameter("b1", [1, H], F32R, isOutput=False)
    if lnwnz:
        lnwd = nc.declare_dram_parameter("lnw", [1, H], F32, isOutput=False)
    if lnbnz:
        lnbd = nc.declare_dram_parameter("lnb", [1, H], F32, isOutput=False)
    wout = nc.declare_dram_parameter("wout", [128, NT * E], F32, isOutput=True)
    gout = nc.declare_dram_parameter("gout", [128, NT], F32, isOutput=True)

    with tile.TileContext(nc) as tc, ExitStack() as ctx:
        singles = ctx.enter_context(tc.tile_pool(name="singles", bufs=1))
        xpool = ctx.enter_context(tc.tile_pool(name="xpool", bufs=4))
        spool = ctx.enter_context(tc.tile_pool(name="spool", bufs=6))
        gpool = ctx.enter_context(tc.tile_pool(name="gpool", bufs=3))
        gtpool = ctx.enter_context(tc.tile_pool(name="gtpool", bufs=2))
        ltpool = ctx.enter_context(tc.tile_pool(name="ltpool", bufs=2))
        tpool = ctx.enter_context(tc.tile_pool(name="tpool", bufs=8))
        h_ps_pool = ctx.enter_context(tc.tile_pool(name="hps", bufs=4, space="PSUM"))
        gt_ps_pool = ctx.enter_context(tc.tile_pool(name="gtps", bufs=2, space="PSUM"))
        lt_ps_pool = ctx.enter_context(tc.tile_pool(name="ltps", bufs=1, space="PSUM"))
        lg_ps_pool = ctx.enter_context(tc.tile_pool(name="lgps", bufs=1, space="PSUM"))

        # ---- constants ----
        w1_sb = singles.tile([128, KD, H], F32R)
        nc.sync.dma_start(w1_sb[:], w1d.ap().rearrange("(k p) h -> p k h", p=128))
        w2_sb = singles.tile([128, KH, E], F32R)
        nc.sync.dma_start(w2_sb[:], w2d.ap().rearrange("(c p) e -> p c e", p=128))
        b2_sb = singles.tile([E, 1], F32)
        nc.sync.dma_start(b2_sb[:], b2d.ap())

        ident_i = singles.tile([128, 128], I32)
        nc.gpsimd.iota(ident_i[:], pattern=[[1, 128]], base=0, channel_multiplier=-1)
        ident = singles.tile([128, 128], F32R)
        nc.vector.tensor_scalar(ident[:], ident_i[:], 0, None, op0=OP.is_equal)

        if b1nz:
            ones_sb = singles.tile([1, TILE], F32R)
            nc.vector.memset(ones_sb[:], 1.0)
            b1_sb = singles.tile([1, H], F32R)
            nc.sync.dma_start(b1_sb[:], b1d.ap())
        if lnwnz:
            lnw_sb = singles.tile([128, H], F32)
            nc.sync.dma_start(lnw_sb[:], lnwd.ap().to_broadcast((128, H)))
        if lnbnz:
            lnb_sb = singles.tile([128, H], F32)
            nc.sync.dma_start(lnb_sb[:], lnbd.ap().to_broadcast((128, H)))

        wout_sb = singles.tile([128, NT * E], F32)
        gap_sb = singles.tile([128, NT], F32)

        xt_ap = xt.ap().rearrange("(k p) t -> p k t", p=128)

        for ig in range(NG):
            t0 = ig * GRP
            xt_g = xpool.tile([128, KD, GRP * TILE], F32R)
            nc.sync.dma_start(
                xt_g[:], xt_ap[:, :, t0 * TILE:(t0 + GRP) * TILE])

            h_tiles = []
            mv_g = spool.tile([128, GRP, 2], F32)
            for t in range(GRP):
                h_ps = h_ps_pool.tile([128, H], F32)
                h_tiles.append(h_ps)
                for k in range(KD):
                    nc.tensor.matmul(
                        h_ps[:],
                        xt_g[:, k, t * TILE:(t + 1) * TILE],
                        w1_sb[:, k, :],
                        start=(k == 0),
                        stop=(k == KD - 1 and not b1nz),
                    )
                if b1nz:
                    nc.tensor.matmul(h_ps[:], ones_sb[:], b1_sb[:],
                                     start=False, stop=True)
                stats = spool.tile([128, 6], F32)
                nc.vector.bn_stats(stats[:], h_ps[:])
                nc.vector.bn_aggr(mv_g[:, t, :], stats[:])

            # rstd = 1/sqrt(var+eps), Newton w/ bit-hack seed, batched [128,GRP,1]
            v_t = spool.tile([128, GRP, 1], F32)
            nc.vector.tensor_scalar(v_t[:], mv_g[:, :, 1:2], LN_EPS, None, op0=OP.add)
            sh = spool.tile([128, GRP, 1], I32)
            nc.vector.tensor_scalar(sh[:], v_t[:].bitcast(I32), 1, None,
                                    op0=OP.logical_shift_right)
            y_t = spool.tile([128, GRP, 1], F32)
            nc.vector.tensor_scalar(y_t[:].bitcast(I32), sh[:], -1, 0x5F3759DF,
                                    op0=OP.mult, op1=OP.add)
            e_t = spool.tile([128, GRP, 1], F32)
            f_t = spool.tile([128, GRP, 1], F32)
            for _ in range(3):
                nc.vector.tensor_tensor(e_t[:], v_t[:], y_t[:], op=OP.mult)
                nc.vector.tensor_tensor(f_t[:], e_t[:], y_t[:], op=OP.mult)
                nc.vector.tensor_scalar(f_t[:], f_t[:], -0.5, 1.5,
                                        op0=OP.mult, op1=OP.add)
                nc.vector.tensor_tensor(y_t[:], y_t[:], f_t[:], op=OP.mult)
            # neg mean * rstd
            nmr = spool.tile([128, GRP, 1], F32)
            nc.vector.scalar_tensor_tensor(
                nmr[:], mv_g[:, :, 0:1], -1.0, y_t[:], op0=OP.mult, op1=OP.mult)

            gt_sb = gtpool.tile([128, KH, GRP * TILE], F32R)
            for t in range(GRP):
                g_sb = gpool.tile([128, H], F32R)
                if not (lnwnz or lnbnz):
                    nc.scalar.activation(g_sb[:], h_tiles[t][:], AF.Gelu,
                                         bias=nmr[:, t, :], scale=y_t[:, t, :])
                else:
                    u_sb = gpool.tile([128, H], F32)
                    nc.vector.tensor_scalar(u_sb[:], h_tiles[t][:],
                                            y_t[:, t, :], nmr[:, t, :],
                                            op0=OP.mult, op1=OP.add)
                    if lnwnz:
                        nc.vector.tensor_tensor(u_sb[:], u_sb[:], lnw_sb[:],
                                                op=OP.mult)
                    if lnbnz:
                        nc.vector.tensor_tensor(u_sb[:], u_sb[:], lnb_sb[:],
                                                op=OP.add)
                    nc.scalar.activation(g_sb[:], u_sb[:], AF.Gelu)
                gt_ps = gt_ps_pool.tile([128, H], F32R)
                for c in range(KH):
                    nc.tensor.transpose(gt_ps[:, c * 128:(c + 1) * 128],
                                        g_sb[:, c * 128:(c + 1) * 128],
                                        ident[:])
                nc.scalar.copy(
                    gt_sb[:, :, t * TILE:(t + 1) * TILE],
                    gt_ps[:].rearrange("p (c q) -> p c q", c=KH))

            lt_ps = lt_ps_pool.tile([E, GRP * TILE], F32)
            for c in range(KH):
                nc.tensor.matmul(lt_ps[:], w2_sb[:, c, :], gt_sb[:, c, :],
                                 start=(c == 0), stop=(c == KH - 1))
            lt_sb = ltpool.tile([E, GRP * TILE], F32R)
            nc.vector.tensor_scalar(lt_sb[:], lt_ps[:], b2_sb[:], None, op0=OP.add)

            for t in range(GRP):
                lg_ps = lg_ps_pool.tile([128, E], F32R)
                nc.tensor.transpose(lg_ps[:], lt_sb[:, t * TILE:(t + 1) * TILE],
                                    ident[0:E, 0:E])
                l_sb = tpool.tile([128, E], F32)
                nc.vector.tensor_copy(l_sb[:], lg_ps[:])
                m1 = tpool.tile([128, 1], F32)
                nc.vector.tensor_reduce(m1[:], l_sb[:], axis=AXX, op=OP.max)
                ismax = tpool.tile([128, E], F32)
                nc.vector.tensor_scalar(ismax[:], l_sb[:], m1[:], None,
                                        op0=OP.is_equal)
                masked = tpool.tile([128, E], F32)
                nc.vector.scalar_tensor_tensor(masked[:], ismax[:], -1e30,
                                               l_sb[:], op0=OP.mult, op1=OP.add)
                m2 = tpool.tile([128, 1], F32)
                nc.vector.tensor_reduce(m2[:], masked[:], axis=AXX, op=OP.max)
                ismax2 = tpool.tile([128, E], F32)
                nc.vector.tensor_scalar(ismax2[:], masked[:], m2[:], None,
                                        op0=OP.is_equal)
                d_t = tpool.tile([128, 1], F32)
                nc.vector.tensor_tensor(d_t[:], m1[:], m2[:], op=OP.subtract)
                masked2 = tpool.tile([128, E], F32)
                nc.vector.scalar_tensor_tensor(masked2[:], ismax2[:], -1e30,
                                               masked[:], op0=OP.mult, op1=OP.add)
                m3 = tpool.tile([128, 1], F32)
                nc.vector.tensor_reduce(m3[:], masked2[:], axis=AXX, op=OP.max)
                th = tpool.tile([128, 1], F32)
                nc.scalar.activation(th[:], d_t[:], AF.Tanh, scale=0.5)
                w1v = tpool.tile([128, 1], F32)
                nc.vector.tensor_scalar(w1v[:], th[:], 0.5, 0.5,
                                        op0=OP.mult, op1=OP.add)
                w2v = tpool.tile([128, 1], F32)
                nc.vector.tensor_scalar(w2v[:], th[:], -0.5, 0.5,
                                        op0=OP.mult, op1=OP.add)
                wtmp = tpool.tile([128, E], F32)
                nc.vector.tensor_scalar(wtmp[:], ismax2[:], w2v[:], None,
                                        op0=OP.mult)
                idx = t0 + t
                nc.vector.tensor_tensor(gap_sb[:, idx:idx + 1], m2[:], m3[:],
                                        op=OP.subtract)
                nc.vector.scalar_tensor_tensor(
                    wout_sb[:, idx * E:(idx + 1) * E], ismax[:], w1v[:],
                    wtmp[:], op0=OP.mult, op1=OP.add)

        nc.sync.dma_start(wout.ap(), wout_sb[:])
        nc.sync.dma_start(gout.ap(), gap_sb[:])

    nc.compile()
    return nc


def _exact_rows(xs, W1, b1, ln_w, ln_b, W2, b2):
    from scipy.special import erf
    h = xs.astype(np.float64) @ W1.astype(np.float64) + b1.astype(np.float64)
    mu = h.mean(1, keepdims=True)
    var = ((h - mu) ** 2).mean(1, keepdims=True)
    hn = (h - mu) / np.sqrt(var + LN_EPS) * ln_w + ln_b
    g = 0.5 * hn * (1.0 + erf(hn / np.sqrt(2.0)))
    lg = g @ W2.astype(np.float64) + b2.astype(np.float64)
    n = len(xs)
    order = np.argsort(-lg, axis=1)
    i1, i2 = order[:, 0], order[:, 1]
    r = np.arange(n)
    e2 = np.exp(lg[r, i2] - lg[r, i1])
    out = np.zeros((n, E), np.float32)
    out[r, i1] = (1.0 / (1.0 + e2)).astype(np.float32)
    out[r, i2] = (e2 / (1.0 + e2)).astype(np.float32)
    return out


def _get(flags):
    if flags not in _cache:
        _cache[flags] = _build(flags)
    return _cache[flags]


def kernel(x, W1, b1, ln_w, ln_b, W2, b2, trace=False):
    x = np.asarray(x, dtype=np.float32)
    W1 = np.asarray(W1, dtype=np.float32)
    b1 = np.asarray(b1, dtype=np.float32)
    ln_w = np.asarray(ln_w, dtype=np.float32)
    ln_b = np.asarray(ln_b, dtype=np.float32)
    W2 = np.asarray(W2, dtype=np.float32)
    b2 = np.asarray(b2, dtype=np.float32)
    assert x.shape == (B, D) and W1.shape == (D, H) and W2.shape == (H, E)

    b1nz = bool(np.any(b1 != 0.0))
    lnwnz = bool(np.any(ln_w != 1.0))
    lnbnz = bool(np.any(ln_b != 0.0))
    flags = (b1nz, lnwnz, lnbnz)
    nc = _get(flags)

    xT = np.ascontiguousarray(x.T)  # [D, B]
    in_maps = []
    for c in range(N_CORES):
        m = {
            "xt": np.ascontiguousarray(xT[:, c * BT:(c + 1) * BT]),
            "w1": W1,
            "w2": W2,
            "b2": b2.reshape(E, 1),
        }
        if b1nz:
            m["b1"] = b1.reshape(1, H)
        if lnwnz:
            m["lnw"] = ln_w.reshape(1, H)
        if lnbnz:
            m["lnb"] = ln_b.reshape(1, H)
        in_maps.append(m)

    res = run_bass_kernel_spmd(nc, in_maps, list(range(N_CORES)), trace=trace)
    shards, gshards = [], []
    for c in range(N_CORES):
        w = res.results[c]["wout"].reshape(128, NT, E)
        shards.append(np.ascontiguousarray(w.transpose(1, 0, 2)).reshape(BT, E))
        gshards.append(np.ascontiguousarray(res.results[c]["gout"].T).reshape(BT))
    weights = np.concatenate(shards, axis=0)
    gaps = np.concatenate(gshards, axis=0)

    # Host repair: rows whose 2nd/3rd logits are within the f32r noise band get
    # recomputed exactly (device result may pick the wrong top-2 set there).
    risky = np.where(gaps < 1e-3)[0]
    if len(risky):
        weights[risky] = _exact_rows(x[risky], W1, b1, ln_w, ln_b, W2, b2)

    f = (weights > 0).astype(np.float32).mean(axis=0)
    P = weights.mean(axis=0)
    lb_loss = np.float32(LB_WEIGHT * E * np.sum(f * P))
    if trace:
        kernel._last_result = res
    return weights, lb_loss
